# revision 35
# baseline (speedup 1.0000x reference)
"""Trainium2 Bass kernel for nn_ConnectLoss (BCE-on-connectivity + edge
min-prob loss + bilateral-voting dice loss).

Strategy: pure data parallel — one batch element per NeuronCore (B=8 on 8
cores). All one-pixel shifts are realized WITHOUT partition-shifted
SBUF->SBUF DMAs (which measure ~20-40us/plane on HW, ~13x the cost model):

  - column (W) shifts: views into zero-padded SBUF tiles (free)
  - row (H) shifts of the target: direct HBM loads with a row offset
  - row shifts of sigmoid planes: HBM round-trip (store + row-offset
    reload) — sequential-pattern DMA runs at full aggregate bandwidth

Inputs are host-cast to bf16: halves HBM traffic and enables the DVE 2x/4x
perf modes for every elementwise op.

Per-core device program reduces everything to a [128, NSTAT] stats tile;
host combines in f64.
"""
import numpy as np
from contextlib import ExitStack

B, CHN, H, W = 8, 8, 512, 512
NCORES = 8
P = 128
NCH = H // P          # 4 row chunks of 128 partitions
WP = W + 4            # padded width; cols 0..1 and 514..515 are zeroed
OFF = 2               # center column offset
# direction d -> (dr, dc): shifted[h, w] = src[h+dr, w+dc], zero outside
DIRS = [(-1, -1), (-1, 0), (-1, 1), (0, -1), (0, 1), (1, -1), (1, 0), (1, 1)]

# sig/x supertile slot order: slots 0..7 hold channels 5,6,7,0,1,2,3,4.
# Slots 0:3 are the sources of the dr=-1 voting shifts (sig7,6,5 for
# d=0,1,2), slots 3:6 of the dr=+1 shifts (sig2,1,0 for d=5,6,7) — each
# group is contiguous both in the supertile and in pred's channel dim.
SLOT_CH = [5, 6, 7, 0, 1, 2, 3, 4]
CH_SLOT = {c: s for s, c in enumerate(SLOT_CH)}

# stats columns (host sums over partitions, cores, and LOG subcolumns)
NSTAT = 16
S_CROSS = 0   # sum_d sum_px conn_d * x_d   (ones-matmul path, partition 0)
S_LOG = 1     # 1..4: sum_px log(1 - sig_d), 2 slots per column
S_LOGPM = 5   # sum_px log(1 - pm)
S_DEN = 6     # sum_px pm
S_T = 7       # sum_px t
S_FIN = 8     # sum_px final
S_FINT = 9    # sum_px final * t
S_CROSS4 = 10  # 10..13: per-chunk <t, Z> dots (PE-Z cross path)

_CACHE: dict = {}


def _emit(tc, pred_ap, tgt_ap, scr_ap, stats_ap, shmat_ap=None,
          grouped_sig=False, gp_edge=True, gp_vote=True, dma_eng="gpsimd",
          pe_cross=False):
    import concourse.bass as bass  # noqa: F401
    from concourse import mybir
    from concourse.tile_rust import add_dep_helper

    nc = tc.nc
    f32, bf16 = mybir.dt.float32, mybir.dt.bfloat16
    Alu = mybir.AluOpType
    Act = mybir.ActivationFunctionType

    with ExitStack() as ctx:
        pers = ctx.enter_context(tc.tile_pool(name="pers", bufs=1))
        sp = ctx.enter_context(tc.tile_pool(name="sp", bufs=1))
        psum_pool = ctx.enter_context(
            tc.tile_pool(name="ps", bufs=1, space="PSUM"))

        x = pers.tile([P, 8, NCH, W], bf16, name="x", tag="x")
        sig = pers.tile([P, 8, NCH, WP], bf16, name="sig", tag="sig")
        svm = pers.tile([P, 3, NCH, WP], bf16, name="svm", tag="svm")
        svp = pers.tile([P, 3, NCH, WP], bf16, name="svp", tag="svp")
        t = pers.tile([P, 3, NCH, WP], bf16, name="t", tag="t")
        stats = pers.tile([P, NSTAT], f32, name="stats", tag="stats")

        nc.vector.memset(stats[:], 0.0)
        # zero all pad columns once per body (views + full-row Ln rely on it)
        for tl, nj in [(sig, 8), (svm, 3), (svp, 3), (t, 3)]:
            nc.vector.memset(tl[:, :, :, 0:OFF], 0.0)
            nc.vector.memset(tl[:, :, :, OFF + W:WP], 0.0)

        def ctr(tl, j):
            return tl[:, j, :, OFF:OFF + W]

        def view(tl, j, dc):
            s0 = OFF + dc
            return tl[:, j, :, s0:s0 + W]

        zrow = pers.tile([1, W], bf16, name="zrow", tag="zrow")
        nc.vector.memset(zrow[:], 0.0)

        def load_row_shifted(dst, j0, nj, src_rows, delta, eng=None):
            """dst[:, j0+k] center <- plane k of src, row-shifted:
            dst[h] = src[h+delta]. src_rows(k, a, b) returns the DRAM AP for
            rows [a, b) of plane k, shaped [b-a, W]. Boundary row zeroed."""
            eng = eng or nc.sync
            per_plane = []
            for k in range(nj):
                j = j0 + k
                insts = []
                per_plane.append(insts)
                if delta == -1:
                    insts.append(eng.dma_start(
                        out=dst[1:P, j, 0:1, OFF:OFF + W],
                        in_=src_rows(k, 0, P - 1).rearrange(
                            "(c p) w -> p c w", c=1)))
                    insts.append(eng.dma_start(
                        out=dst[:, j, 1:NCH, OFF:OFF + W],
                        in_=src_rows(k, P - 1, H - 1).rearrange(
                            "(c p) w -> p c w", p=P)))
                    nc.vector.memset(dst[0:1, j, 0:1, OFF:OFF + W], 0.0)
                else:
                    insts.append(eng.dma_start(
                        out=dst[:, j, 0:NCH - 1, OFF:OFF + W],
                        in_=src_rows(k, 1, 1 + P * (NCH - 1)).rearrange(
                            "(c p) w -> p c w", p=P)))
                    insts.append(eng.dma_start(
                        out=dst[0:P - 1, j, NCH - 1:NCH, OFF:OFF + W],
                        in_=src_rows(k, 1 + P * (NCH - 1), H).rearrange(
                            "(c p) w -> p c w", c=1)))
                    insts.append(eng.dma_start(
                        out=dst[P - 1:P, j, NCH - 1, OFF:OFF + W],
                        in_=zrow[:]))
            return per_plane

        # ---- target: t0 + row-shifted copies straight from HBM ------------
        nc.sync.dma_start(
            out=ctr(t, 0), in_=tgt_ap.rearrange("(c p) w -> p c w", p=P))
        load_row_shifted(t, 1, 1, lambda k, a, b: tgt_ap[a:b], -1)
        load_row_shifted(t, 2, 1, lambda k, a, b: tgt_ap[a:b], 1)

        # ---- pred loads (3 groups) + grouped sigmoids ---------------------
        # slots 0:3 = pred[5:8], slots 3:6 = pred[0:3], slots 6:8 = pred[3:5]
        groups = [(0, 3, 5), (3, 3, 0), (6, 2, 3)]
        sig_insts = []
        for (s0, n, c0) in groups:
            nc.sync.dma_start(
                out=x[:, s0:s0 + n, :, :],
                in_=pred_ap[c0:c0 + n].rearrange("j (c p) w -> p j c w", p=P))
            if grouped_sig:
                sig_insts.append(nc.scalar.activation(
                    sig[:, s0:s0 + n, :, OFF:OFF + W], x[:, s0:s0 + n, :, :],
                    Act.Sigmoid))
            else:
                for s in range(s0, s0 + n):
                    sig_insts.append(nc.scalar.activation(
                        ctr(sig, s), x[:, s, :, :], Act.Sigmoid))

        # ---- sigmoid row-shifts via HBM round-trip ------------------------
        deng = nc.gpsimd if dma_eng == "gpsimd" else nc.sync
        stores = []
        for j in range(6):
            stores.append(deng.dma_start(
                out=scr_ap[j].rearrange("(c p) w -> p c w", p=P),
                in_=sig[:, j, :, OFF:OFF + W]))
        rl_m = load_row_shifted(svm, 0, 3,
                                lambda k, a, b: scr_ap[k, a:b], -1, eng=deng)
        rl_p = load_row_shifted(svp, 0, 3,
                                lambda k, a, b: scr_ap[3 + k, a:b], 1,
                                eng=deng)
        for k, plane in enumerate(rl_m):
            for ins in plane:
                add_dep_helper(ins.ins, stores[k].ins, sync=True,
                               reason="DRAM RAW: reload after store")
        for k, plane in enumerate(rl_p):
            for ins in plane:
                add_dep_helper(ins.ins, stores[3 + k].ins, sync=True,
                               reason="DRAM RAW: reload after store")

        # ---- edge mask (neighbor-count trick) -----------------------------
        # S8 = sum_d shift_d(t) = (A(-1)+A(+1)) + B with A = t0+tm+tp,
        # B = tm+tp;  U = t * S8;  edge&t mask e1 = (|U-4| <= 3.5)
        t0c = ctr(t, 0)
        Bs = sp.tile([P, NCH, W], bf16, name="Bs", tag="Bs")
        Ap = sp.tile([P, NCH, WP], bf16, name="Ap", tag="Ap")
        nc.vector.memset(Ap[:, :, 0:OFF], 0.0)
        nc.vector.memset(Ap[:, :, OFF + W:WP], 0.0)
        S8 = sp.tile([P, NCH, W], bf16, name="S8", tag="S8")
        e1 = sp.tile([P, NCH, W], bf16, name="e1", tag="e1")
        edge_eng = nc.gpsimd if gp_edge else nc.vector
        edge_eng.tensor_tensor(Bs[:], ctr(t, 1), ctr(t, 2), Alu.add)
        edge_eng.tensor_tensor(Ap[:, :, OFF:OFF + W], Bs[:], t0c, Alu.add)
        nc.vector.tensor_tensor(S8[:], Ap[:, :, OFF - 1:OFF - 1 + W],
                                Ap[:, :, OFF + 1:OFF + 1 + W], Alu.add)
        nc.vector.tensor_tensor(S8[:], S8[:], Bs[:], Alu.add)
        nc.vector.tensor_tensor(S8[:], S8[:], t0c, Alu.mult)
        e2 = sp.tile([P, NCH, W], bf16, name="e2", tag="e2")
        nc.vector.tensor_scalar(e1[:], S8[:], 7.5, None, Alu.is_lt)
        (nc.gpsimd if gp_edge else nc.vector).tensor_scalar(
            e2[:], S8[:], 0.5, None, Alu.is_gt)
        nc.vector.tensor_tensor(e1[:], e1[:], e2[:], Alu.mult)

        # ---- cross = sum_d <t * shift_d(t), x_d> --------------------------
        if not pe_cross:
            # DVE+PE ones-matmul path: conn_d = t*st_d, wmul = conn*x
            ones = pers.tile([P, 1], bf16, name="ones", tag="ones")
            nc.vector.memset(ones[:], 1.0)
            ps_cross = psum_pool.tile([1, W], f32, name="psc", tag="psc")
            conn = sp.tile([P, NCH, W], bf16, name="conn", tag="conn")
            wmul = sp.tile([P, NCH, W], bf16, name="wmul", tag="wmul")
            mm_idx, mm_total = 0, 8 * NCH
            for d, (dr, dc) in enumerate(DIRS):
                tj = {-1: 1, 0: 0, 1: 2}[dr]
                s = CH_SLOT[d]
                nc.vector.tensor_tensor(conn[:], t0c, view(t, tj, dc),
                                        Alu.mult)
                nc.vector.tensor_tensor(wmul[:], conn[:], x[:, s, :, :],
                                        Alu.mult)
                for c in range(NCH):
                    nc.tensor.matmul(ps_cross[:], ones[:], wmul[:, c, :],
                                     start=(mm_idx == 0),
                                     stop=(mm_idx == mm_total - 1))
                    mm_idx += 1
            nc.vector.tensor_reduce(out=stats[0:1, S_CROSS:S_CROSS + 1],
                                    in_=ps_cross[:],
                                    axis=mybir.AxisListType.X, op=Alu.add)
        else:
            # PE-Z path: y_d = t*x_d (in place over x), then PE accumulates
            # Z = sum_d shift_{-d}(y_d) in PSUM via shift matmuls, and
            # cross = <t, Z> as 4 per-chunk fused dots.
            # shmat: [5,128,128]: I, M+1 (out[q]=in[q+1]), M-1, Eup (out
            # row 127 = in row 0), Edn (out row 0 = in row 127).
            sh = pers.tile([P, 5, P], bf16, name="sh", tag="sh")
            nc.sync.dma_start(out=sh[:],
                              in_=shmat_ap.rearrange("k p q -> p k q"))
            for d in range(8):
                s = CH_SLOT[d]
                nc.vector.tensor_tensor(x[:, s, :, :], t0c, x[:, s, :, :],
                                        Alu.mult)

            Z = psum_pool.tile([P, NCH, W], f32, name="Z", tag="Z")

            def colrange(dc):
                # out[w] = y[w - dc]
                if dc == 1:
                    return slice(1, W), slice(0, W - 1)
                if dc == -1:
                    return slice(0, W - 1), slice(1, W)
                return slice(0, W), slice(0, W)

            MAT = {1: 1, -1: 2}   # delta -> shmat row (0 = identity)
            for c in range(NCH):
                mms = []
                # d=1 (dr=-1 -> delta=+1, dc=0): full width, starts the bank
                order = [1, 0, 2, 3, 4, 5, 6, 7]
                for d in order:
                    dr, dc = DIRS[d]
                    delta = -dr
                    ocols, rcols = colrange(dc)
                    s = CH_SLOT[d]
                    mms.append((sh[:, MAT.get(delta, 0), :],
                                x[:, s, c, rcols], Z[:, c, ocols]))
                    if delta == 1 and c < NCH - 1:
                        mms.append((sh[:, 3, :], x[:, s, c + 1, rcols],
                                    Z[:, c, ocols]))
                    elif delta == -1 and c > 0:
                        mms.append((sh[:, 4, :], x[:, s, c - 1, rcols],
                                    Z[:, c, ocols]))
                for i, (lhsT, rhs, out) in enumerate(mms):
                    nc.tensor.matmul(out, lhsT, rhs, start=(i == 0),
                                     stop=(i == len(mms) - 1))
            scrapb = sp.tile([P, W], bf16, name="scrapb", tag="scrapb")
            for c in range(NCH):
                nc.vector.tensor_tensor_reduce(
                    out=scrapb[:], in0=t[:, 0, c, OFF:OFF + W],
                    in1=Z[:, c, :], scale=1.0, scalar=0.0,
                    op0=Alu.mult, op1=Alu.add,
                    accum_out=stats[:, S_CROSS4 + c:S_CROSS4 + c + 1])

        # ---- pm = min_d sig_d * e1; seg-loss sums -------------------------
        smin = sp.tile([P, NCH, W], bf16, name="smin", tag="smin")
        nc.vector.tensor_tensor(smin[:], ctr(sig, 0), ctr(sig, 1), Alu.min)
        for s in range(2, 8):
            nc.vector.tensor_tensor(smin[:], smin[:], ctr(sig, s), Alu.min)
        pm = sp.tile([P, NCH, W], bf16, name="pm", tag="pm")
        nc.vector.tensor_tensor(pm[:], smin[:], e1[:], Alu.mult)
        dump = sp.tile([P, NCH, W], bf16, name="dump", tag="dump")
        dump2 = sp.tile([P, NCH, W], bf16, name="dump2", tag="dump2")
        nc.vector.tensor_scalar(dump[:], pm[:], 1.0, None, Alu.mult, Alu.add,
                                accum_out=stats[:, S_DEN:S_DEN + 1])
        nc.vector.tensor_scalar(dump2[:], t0c, 1.0, None, Alu.mult, Alu.add,
                                accum_out=stats[:, S_T:S_T + 1])

        # ---- bilateral voting ---------------------------------------------
        # vote_d = sig_d * shift_{DIRS[d]}(sig_{7-d}); sv planes:
        # d=0,1,2 -> svm[2,1,0] with dc=-1,0,+1 ; d=5,6,7 -> svp[2,1,0]
        # with dc=-1,0,+1 ; d=3,4 -> views of sig slots 7,6.
        sv_views = {
            0: view(svm, 2, -1), 1: view(svm, 1, 0), 2: view(svm, 0, 1),
            3: view(sig, 7, -1), 4: view(sig, 6, 1),
            5: view(svp, 2, -1), 6: view(svp, 1, 0), 7: view(svp, 0, 1),
        }
        # d=3,4 need no reloads (pure column shifts): compute their products
        # and max on GPSIMD in parallel with the DVE tree over the rest
        vg1 = sp.tile([P, NCH, W], bf16, name="vg1", tag="vg1")
        vg2 = sp.tile([P, NCH, W], bf16, name="vg2", tag="vg2")
        vote_eng = nc.gpsimd if gp_vote else nc.vector
        vote_eng.tensor_tensor(vg1[:], ctr(sig, CH_SLOT[3]), sv_views[3],
                               Alu.mult)
        vote_eng.tensor_tensor(vg2[:], ctr(sig, CH_SLOT[4]), sv_views[4],
                               Alu.mult)

        vacc = sp.tile([P, NCH, W], bf16, name="vacc", tag="vacc")
        vtmp = sp.tile([P, NCH, W], bf16, name="vtmp", tag="vtmp")
        nc.vector.tensor_tensor(vacc[:], ctr(sig, CH_SLOT[0]), sv_views[0],
                                Alu.mult)
        for d in (1, 2, 5, 6, 7):
            nc.vector.tensor_tensor(vtmp[:], ctr(sig, CH_SLOT[d]),
                                    sv_views[d], Alu.mult)
            nc.vector.tensor_tensor(vacc[:], vacc[:], vtmp[:], Alu.max)
        nc.vector.tensor_tensor(vacc[:], vacc[:], vg1[:], Alu.max)
        nc.vector.tensor_tensor(vacc[:], vacc[:], vg2[:], Alu.max)
        nc.vector.tensor_scalar(dump[:], vacc[:], 1.0, None, Alu.mult,
                                Alu.add, accum_out=stats[:, S_FIN:S_FIN + 1])
        nc.vector.tensor_tensor(vtmp[:], vacc[:], t0c, Alu.mult)
        nc.vector.tensor_scalar(dump[:], vtmp[:], 1.0, None, Alu.mult,
                                Alu.add, accum_out=stats[:, S_FINT:S_FINT + 1])

        # ---- Ln phase (after all sigmoids: one act-table switch) ----------
        # full padded rows are safe: pads are 0 -> Ln(1-0) = 0
        lout = sp.tile([P, 2, NCH, WP], bf16, name="lout", tag="lout")
        last_sig = sig_insts[-1]
        for i in range(4):
            ins = nc.scalar.activation(
                lout[:], sig[:, 2 * i:2 * i + 2, :, :], Act.Ln,
                bias=1.0, scale=-1.0,
                accum_out=stats[:, S_LOG + i:S_LOG + i + 1])
            add_dep_helper(ins.ins, last_sig.ins, sync=False,
                           reason="batch act-table: Ln after all sigmoids")
        ins = nc.scalar.activation(
            lout[:, 0, :, 0:W], pm[:], Act.Ln, bias=1.0, scale=-1.0,
            accum_out=stats[:, S_LOGPM:S_LOGPM + 1])
        add_dep_helper(ins.ins, last_sig.ins, sync=False,
                       reason="batch act-table: Ln after all sigmoids")

        nc.sync.dma_start(out=stats_ap, in_=stats[:])


def _build_nc(repeat=1, **kw):
    import concourse.bacc as bacc
    import concourse.tile as tile
    from concourse import mybir

    nc = bacc.Bacc("TRN2", target_bir_lowering=False, debug=False,
                   enable_asserts=False, num_devices=NCORES)
    f32, bf16 = mybir.dt.float32, mybir.dt.bfloat16
    pred_t = nc.dram_tensor("pred", [CHN, H, W], bf16, kind="ExternalInput")
    tgt_t = nc.dram_tensor("target", [H, W], bf16, kind="ExternalInput")
    shmat_t = nc.dram_tensor("shmat", [5, P, P], bf16, kind="ExternalInput")
    scr_t = nc.dram_tensor("scr", [6, H, W], bf16, kind="Internal")
    stats_t = nc.dram_tensor("stats", [P, NSTAT], f32, kind="ExternalOutput")
    with tile.TileContext(nc) as tc:
        for _ in range(repeat):
            _emit(tc, pred_t.ap(), tgt_t.ap(), scr_t.ap(), stats_t.ap(),
                  shmat_ap=shmat_t.ap(), **kw)
    nc.compile()
    return nc


def _get_nc():
    if "nc" not in _CACHE:
        _CACHE["nc"] = _build_nc()
    return _CACHE["nc"]


def _shift_mats():
    import ml_dtypes
    bf = ml_dtypes.bfloat16
    m = np.zeros((5, P, P), np.float32)
    m[0] = np.eye(P)
    m[1, np.arange(1, P), np.arange(P - 1)] = 1.0   # out[q] = in[q+1]
    m[2, np.arange(P - 1), np.arange(1, P)] = 1.0   # out[q] = in[q-1]
    m[3, 0, P - 1] = 1.0                            # out[127] = in_next[0]
    m[4, P - 1, 0] = 1.0                            # out[0] = in_prev[127]
    return m.astype(bf)


def _make_in_maps(pred, target):
    import ml_dtypes
    bf = ml_dtypes.bfloat16
    pred_b = np.asarray(pred, dtype=np.float32).astype(bf)
    tgt_b = np.asarray(target, dtype=np.float32).astype(bf)
    sh = _shift_mats()
    return [{"pred": np.ascontiguousarray(pred_b[b]),
             "target": np.ascontiguousarray(tgt_b[b, 0]),
             "shmat": sh} for b in range(B)]


def _combine(stats_list):
    s = np.stack([np.asarray(sl).astype(np.float64) for sl in stats_list])
    cols = s.sum(axis=1)                                      # [B, NSTAT]
    cross = cols[:, S_CROSS].sum() + cols[:, S_CROSS4:S_CROSS4 + 4].sum()
    slog = cols[:, S_LOG:S_LOG + 4].sum()
    slogpm = cols[:, S_LOGPM].sum()
    den = cols[:, S_DEN].sum()
    sum_t = cols[:, S_T]
    sum_fin = cols[:, S_FIN]
    sum_fint = cols[:, S_FINT]

    n_elem = B * CHN * H * W
    conn_loss = (-slog - cross) / n_elem
    edge_loss = -slogpm / den
    dice = (2.0 * sum_fint + 1.0) / (sum_fin + sum_t + 1.0)
    seg_loss = (1.0 - dice).mean()
    return np.asarray(conn_loss + edge_loss + seg_loss, dtype=np.float32)


def _is_shift_mats(hori, verti):
    hm = np.zeros((W, W), np.float32)
    hm[np.arange(W - 1), np.arange(1, W)] = 1.0
    vm = np.zeros((H, H), np.float32)
    vm[np.arange(H - 1), np.arange(1, H)] = 1.0
    return (np.array_equal(np.asarray(hori),
                           np.broadcast_to(hm, (B, 1, W, W))) and
            np.array_equal(np.asarray(verti),
                           np.broadcast_to(vm, (B, 1, H, H))))


def kernel(pred, target, hori_translation, verti_translation):
    pred = np.asarray(pred, dtype=np.float32)
    target = np.asarray(target, dtype=np.float32)
    if not _is_shift_mats(hori_translation, verti_translation):
        return _fallback(pred, target,
                         np.asarray(hori_translation, dtype=np.float32),
                         np.asarray(verti_translation, dtype=np.float32))

    from concourse.bass_utils import run_bass_kernel_spmd
    nc = _get_nc()
    res = run_bass_kernel_spmd(nc, _make_in_maps(pred, target),
                               list(range(NCORES)))
    return _combine([res.results[b]["stats"] for b in range(B)])


# ---------------------------------------------------------------------------
# Fallback for non-shift translation matrices: faithful numpy replica of the
# reference (never taken for the standard setup_inputs data).
def _fallback(pred, target, hori, verti):
    NEG_CLAMP = -100.0
    dt = np.float64
    predd, targetd = pred.astype(dt), target.astype(dt)
    horid, vertid = hori.astype(dt), verti.astype(dt)

    z = np.zeros_like(targetd)
    def sh(dr, dc):
        out = z.copy()
        hs = slice(max(0, -dr), H - max(0, dr))
        ws = slice(max(0, -dc), W - max(0, dc))
        hsrc = slice(max(0, dr), H + min(0, dr) if dr < 0 else H)
        wsrc = slice(max(0, dc), W + min(0, dc) if dc < 0 else W)
        out[..., hs, ws] = targetd[..., hsrc, wsrc]
        return out

    conn_t = np.stack([targetd * sh(dr, dc) for (dr, dc) in DIRS], axis=2)
    sigd = 1.0 / (1.0 + np.exp(-predd))
    with np.errstate(divide="ignore"):
        lp = np.maximum(np.log(sigd), NEG_CLAMP)
        l1p = np.maximum(np.log1p(-sigd), NEG_CLAMP)
    ct = conn_t.reshape(predd.shape)
    conn_loss = (-(ct * lp + (1.0 - ct) * l1p)).mean()

    sum_conn = conn_t.sum(axis=2)
    edge = ((sum_conn < 8) & (sum_conn > 0)).astype(dt)
    sig5 = sigd.reshape(B, 1, 8, H, W)
    pmin = np.min(sig5, axis=2) * edge
    edge_loss = (-np.maximum(np.log1p(-pmin), NEG_CLAMP)).sum() / pmin.sum()

    mm_h = lambda m, T: np.einsum('bchw,bcwv->bchv', m, T)
    mm_hT = lambda m, T: np.einsum('bchw,bcvw->bchv', m, T)
    mm_v = lambda T, m: np.einsum('bcrh,bchw->bcrw', T, m)
    mm_vT = lambda T, m: np.einsum('bchr,bchw->bcrw', T, m)
    c = sig5
    right = mm_h(c[:, :, 4], horid)
    left = mm_hT(c[:, :, 3], horid)
    bottom = mm_vT(vertid, c[:, :, 6])
    up = mm_v(vertid, c[:, :, 1])
    left_bottom = mm_hT(mm_vT(vertid, c[:, :, 5]), horid)
    right_above = mm_h(mm_v(vertid, c[:, :, 2]), horid)
    left_above = mm_hT(mm_v(vertid, c[:, :, 0]), horid)
    right_bottom = mm_h(mm_vT(vertid, c[:, :, 7]), horid)
    vote = np.stack([c[:, :, 0] * right_bottom, c[:, :, 1] * bottom,
                     c[:, :, 2] * left_bottom, c[:, :, 3] * right,
                     c[:, :, 4] * left, c[:, :, 5] * right_above,
                     c[:, :, 6] * up, c[:, :, 7] * left_above], axis=2)
    final_pred = vote.max(axis=2)
    inter = (final_pred * targetd).sum(axis=(2, 3))
    union = final_pred.sum(axis=(2, 3)) + targetd.sum(axis=(2, 3))
    dice = (2.0 * inter + 1.0) / (union + 1.0)
    seg_loss = (1.0 - dice).mean()
    return np.asarray(conn_loss + edge_loss + seg_loss, dtype=np.float32)
